# revision 5
# baseline (speedup 1.0000x reference)
"""Distributed GQA attention prefill kernel for one TRN2 chip (8 NeuronCores).

Sharding: tensor-parallel over heads (4-way) x data-parallel over batch (2-way).
Core c handles batch b=c//4, TP rank r=c%4 (8 q-heads, 2 kv-heads each).
Device-side: QKV projections (fp32r matmuls), RoPE (partition-swap matmul +
DVE), causal flash-style attention in a transposed layout (scores^T so softmax
sums come from a ones-matmul and no transposes are ever needed), output
projection, then a row-blocked ReduceScatter(add) over each TP group.
Host-side: input sharding/transpose prep and final reassembly only.
"""

import os
import sys
import numpy as np

B, S, D = 2, 2048, 4096
H, KV, HD = 32, 8, 128
TP = 4
QH = H // TP          # 8 q heads per core
G = KV // TP          # 2 kv heads per core
P = 128
QT = 512              # q-tile (free dim)
NQT = S // QT         # 4
NDC = 4               # D chunks of 1024 for QKV accumulation
SCALE = float(HD) ** -0.5

LAST_EXEC_NS = None
LAST_TRACE_DIR = None


def _build():
    sys.path.insert(0, "/opt/trn_rl_repo")
    import concourse.bass as bass
    from concourse import bacc
    import concourse.mybir as mybir
    import concourse.tile as tile
    from contextlib import ExitStack

    F32 = mybir.dt.float32
    F32R = mybir.dt.float32r
    Exp = mybir.ActivationFunctionType.Exp
    Copy = mybir.ActivationFunctionType.Copy
    MUL = mybir.AluOpType.mult
    ADD = mybir.AluOpType.add

    nc = bacc.Bacc(None, target_bir_lowering=False)
    xt_e = nc.dram_tensor("xt", [D, S], F32R, kind="ExternalInput")
    wq_e = nc.dram_tensor("wq", [D, QH * HD], F32R, kind="ExternalInput")
    wk_e = nc.dram_tensor("wk", [D, G * HD], F32R, kind="ExternalInput")
    wv_e = nc.dram_tensor("wv", [D, G * HD], F32R, kind="ExternalInput")
    wo_e = nc.dram_tensor("wo", [QH * HD, D], F32R, kind="ExternalInput")
    cost_e = nc.dram_tensor("cost", [P, S], F32R, kind="ExternalInput")
    sint_e = nc.dram_tensor("sint", [P, S], F32R, kind="ExternalInput")
    mbig_e = nc.dram_tensor("mbig", [P, 1024], F32R, kind="ExternalInput")
    onec_e = nc.dram_tensor("onec", [P, 1], F32R, kind="ExternalInput")
    oner_e = nc.dram_tensor("oner", [1, P], F32R, kind="ExternalInput")
    pswap_e = nc.dram_tensor("pswap", [P, P], F32R, kind="ExternalInput")
    NO_CC = os.environ.get("KERNEL_NO_CC", "0") == "1"
    out_shape = [S, D] if NO_CC else [NQT * P, D]
    out_e = nc.dram_tensor("out", out_shape, F32, kind="ExternalOutput")

    with ExitStack() as top:
        top.enter_context(nc.allow_low_precision(reason="fp32r attention"))
        tc = top.enter_context(tile.TileContext(nc))
        const = top.enter_context(tc.tile_pool(name="const", bufs=1))
        mbig = const.tile([P, 1024], F32R)
        nc.sync.dma_start(mbig[:], mbig_e[:])
        onec = const.tile([P, 1], F32R)
        nc.sync.dma_start(onec[:], onec_e[:])
        oner = const.tile([1, P], F32R)
        nc.sync.dma_start(oner[:], oner_e[:])

        pers = top.enter_context(tc.tile_pool(name="pers", bufs=1))
        qT = [pers.tile([P, S], F32R, name=f"qT{h}") for h in range(QH)]
        kT = [pers.tile([P, S], F32R, name=f"kT{g}") for g in range(G)]

        dram = top.enter_context(tc.tile_pool(name="dram", bufs=1, space="DRAM"))
        vdram = dram.tile([S, G * HD], F32R)
        parts = [dram.tile([QT, D], F32, name=f"part{t}") for t in range(NQT)]
        ccouts = [dram.tile([P, D], F32, name=f"ccout{t}") for t in range(NQT)]

        # ---------------- phase 1: QKV projections ----------------
        with tc.tile_pool(name="xtp", bufs=2) as xt_pool, \
             tc.tile_pool(name="wqp", bufs=1) as wq_pool, \
             tc.tile_pool(name="wkvp", bufs=1) as wkv_pool, \
             tc.tile_pool(name="vsbp", bufs=1) as vsb_pool, \
             tc.tile_pool(name="ps1", bufs=4, space="PSUM") as ps1:
            vsb = vsb_pool.tile([P, S // P, G * HD], F32R)

            for c in range(NDC):
                d0 = c * 1024
                wk_t = wkv_pool.tile([P, 8, G * HD], F32R, name="wk_t")
                nc.sync.dma_start(
                    wk_t[:], wk_e[d0:d0 + 1024, :].rearrange("(n p) m -> p n m", p=P))
                wv_t = wkv_pool.tile([P, 8, G * HD], F32R, name="wv_t")
                nc.sync.dma_start(
                    wv_t[:], wv_e[d0:d0 + 1024, :].rearrange("(n p) m -> p n m", p=P))
                wq_ts = []
                for hp in range(4):
                    wq_t = wq_pool.tile([P, 8, 2 * HD], F32R, name=f"wq_t{hp}")
                    nc.sync.dma_start(
                        wq_t[:],
                        wq_e[d0:d0 + 1024, hp * 256:(hp + 1) * 256].rearrange(
                            "(n p) m -> p n m", p=P))
                    wq_ts.append(wq_t)

                for t in range(NQT):
                    s0 = t * QT
                    xt_t = xt_pool.tile([P, 8, QT], F32R)
                    nc.sync.dma_start(
                        xt_t[:],
                        xt_e[d0:d0 + 1024, s0:s0 + QT].rearrange(
                            "(n p) s -> p n s", p=P))
                    for h in range(QH):
                        ps = ps1.tile([P, QT], F32, tag="qkv")
                        w = wq_ts[h // 2]
                        c0 = (h % 2) * HD
                        for dk in range(8):
                            nc.tensor.matmul(
                                ps[:], w[:, dk, c0:c0 + HD], xt_t[:, dk, :],
                                start=(dk == 0), stop=(dk == 7))
                        dst = qT[h][:, s0:s0 + QT]
                        if c == 0:
                            nc.scalar.activation(dst, ps[:], Copy)
                        else:
                            nc.vector.tensor_tensor(dst, dst, ps[:], ADD)
                    for g in range(G):
                        ps = ps1.tile([P, QT], F32, tag="qkv")
                        for dk in range(8):
                            nc.tensor.matmul(
                                ps[:], wk_t[:, dk, g * HD:(g + 1) * HD],
                                xt_t[:, dk, :],
                                start=(dk == 0), stop=(dk == 7))
                        dst = kT[g][:, s0:s0 + QT]
                        if c == 0:
                            nc.scalar.activation(dst, ps[:], Copy)
                        else:
                            nc.vector.tensor_tensor(dst, dst, ps[:], ADD)
                    for sub in range(4):
                        ps = ps1.tile([P, G * HD], F32, tag="vps", bufs=2)
                        for dk in range(8):
                            nc.tensor.matmul(
                                ps[:], xt_t[:, dk, sub * P:(sub + 1) * P],
                                wv_t[:, dk, :],
                                start=(dk == 0), stop=(dk == 7))
                        dst = vsb[:, t * 4 + sub, :]
                        if c == 0:
                            nc.scalar.activation(dst, ps[:], Copy)
                        else:
                            nc.vector.tensor_tensor(dst, dst, ps[:], ADD)

            nc.sync.dma_start(
                vdram[:].rearrange("(n p) m -> p n m", p=P), vsb[:])

        # ---------------- phase 1b: RoPE (in place on qT/kT) ----------------
        with tc.tile_pool(name="trig", bufs=1) as trig_pool, \
             tc.tile_pool(name="ptmp", bufs=3) as ptmp_pool, \
             tc.tile_pool(name="psr", bufs=2, space="PSUM") as psr:
            cosT = trig_pool.tile([P, S], F32R)
            nc.sync.dma_start(cosT[:], cost_e[:])
            sinT = trig_pool.tile([P, S], F32R)
            nc.sync.dma_start(sinT[:], sint_e[:])
            pswap = trig_pool.tile([P, P], F32R)
            nc.sync.dma_start(pswap[:], pswap_e[:])
            for lst in (qT, kT):
                for tile_ in lst:
                    for t in range(NQT):
                        sl = slice(t * QT, (t + 1) * QT)
                        ps = psr.tile([P, QT], F32, tag="rope")
                        nc.tensor.matmul(ps[:], pswap[:], tile_[:, sl],
                                         start=True, stop=True)
                        tmp = ptmp_pool.tile([P, QT], F32R, tag="rtmp")
                        nc.vector.tensor_tensor(tmp[:], ps[:], sinT[:, sl], MUL)
                        nc.vector.tensor_tensor(tile_[:, sl], tile_[:, sl],
                                                cosT[:, sl], MUL)
                        nc.vector.tensor_tensor(tile_[:, sl], tile_[:, sl],
                                                tmp[:], ADD)

        # ---------------- phase 2+3: attention + output projection ----------------
        with tc.tile_pool(name="attn", bufs=1) as attn_pool, \
             tc.tile_pool(name="probs", bufs=3) as probs_pool, \
             tc.tile_pool(name="vk", bufs=1) as vk_pool, \
             tc.tile_pool(name="rp", bufs=1) as rp_pool, \
             tc.tile_pool(name="wop", bufs=2) as wo_pool, \
             tc.tile_pool(name="pss", bufs=2, space="PSUM") as pss, \
             tc.tile_pool(name="pspv", bufs=2, space="PSUM") as pspv, \
             tc.tile_pool(name="pssum", bufs=2, space="PSUM") as pssum, \
             tc.tile_pool(name="pswo", bufs=2, space="PSUM") as pswo:
            attnT = [attn_pool.tile([P, S], F32R, name=f"attnT{h}")
                     for h in range(QH)]
            for t in range(NQT):
                q0 = t * QT
                nk = 4 * (t + 1)
                vks = []
                for g in range(G):
                    vk = vk_pool.tile([P, 16, HD], F32R, tag=f"vk{g}")
                    nc.sync.dma_start(
                        vk[:, :nk, :],
                        vdram[:nk * P, g * HD:(g + 1) * HD].rearrange(
                            "(n p) m -> p n m", p=P))
                    vks.append(vk)
                for h in range(QH):
                    g = h // 4
                    pv = pspv.tile([P, QT], F32, tag="pv")
                    sm = pssum.tile([1, QT], F32, tag="sm")
                    for ki in range(nk):
                        k0 = ki * P
                        ps_s = pss.tile([P, QT], F32, tag="s")
                        nc.tensor.matmul(
                            ps_s[:], kT[g][:, k0:k0 + P],
                            qT[h][:, q0:q0 + QT], start=True, stop=True)
                        pr = probs_pool.tile([P, QT], F32R, tag="pr")
                        nc.scalar.activation(pr[:], ps_s[:], Exp, scale=SCALE)
                        if ki >= nk - 4:
                            off = k0 - q0
                            nc.vector.tensor_tensor(
                                pr[:], pr[:], mbig[:, 512 - off:1024 - off], MUL)
                        nc.tensor.matmul(pv[:], vks[g][:, ki, :], pr[:],
                                         start=(ki == 0), stop=(ki == nk - 1))
                        nc.tensor.matmul(sm[:], onec[:], pr[:],
                                         start=(ki == 0), stop=(ki == nk - 1))
                    recip = rp_pool.tile([1, QT], F32R, tag="recip")
                    nc.vector.reciprocal(recip[:], sm[:])
                    ps_b = pss.tile([P, QT], F32, tag="s")
                    nc.tensor.matmul(ps_b[:], oner[:], recip[:],
                                     start=True, stop=True)
                    dst = attnT[h][:, q0:q0 + QT]
                    nc.scalar.activation(dst, pv[:], Copy)
                    nc.vector.tensor_tensor(dst, dst, ps_b[:], MUL)

                # output projection for this q-tile
                for n in range(8):
                    n0 = n * QT
                    wo_a = wo_pool.tile([P, 4, QT], F32R, tag="wo")
                    nc.sync.dma_start(
                        wo_a[:], wo_e[0:512, n0:n0 + QT].rearrange(
                            "(a p) m -> p a m", p=P))
                    wo_b = wo_pool.tile([P, 4, QT], F32R, tag="wo")
                    nc.sync.dma_start(
                        wo_b[:], wo_e[512:1024, n0:n0 + QT].rearrange(
                            "(a p) m -> p a m", p=P))
                    for si in range(4):
                        s0 = q0 + si * P
                        ps_o = pswo.tile([P, QT], F32, tag="wo")
                        for hh in range(QH):
                            w = wo_a if hh < 4 else wo_b
                            nc.tensor.matmul(
                                ps_o[:], attnT[hh][:, s0:s0 + P],
                                w[:, hh % 4, :],
                                start=(hh == 0), stop=(hh == QH - 1))
                        osb = probs_pool.tile([P, QT], F32, tag="pr")
                        nc.scalar.activation(osb[:], ps_o[:], Copy)
                        nc.sync.dma_start(
                            parts[t][si * P:(si + 1) * P, n0:n0 + QT], osb[:])

                if NO_CC:
                    nc.sync.dma_start(out_e[t * QT:(t + 1) * QT, :], parts[t][:])
                else:
                    nc.gpsimd.collective_compute(
                        "ReduceScatter", ADD,
                        replica_groups=[[0, 1, 2, 3], [4, 5, 6, 7]],
                        ins=[parts[t].opt()], outs=[ccouts[t].opt()])
                    nc.sync.dma_start(out_e[t * P:(t + 1) * P, :], ccouts[t][:])

    nc.compile()
    return nc


def _prep_in_maps(x, wq, wk, wv, wo, cos, sin):
    cosT = np.empty((HD, S), np.float32)
    sinT = np.empty((HD, S), np.float32)
    cosT[0::2] = cos.T
    cosT[1::2] = cos.T
    sinT[0::2] = -sin.T
    sinT[1::2] = sin.T
    mbig = (np.arange(1024)[None, :] >= (np.arange(P)[:, None] + 512)
            ).astype(np.float32)
    onec = np.ones((P, 1), np.float32)
    oner = np.ones((1, P), np.float32)
    pswap = np.zeros((P, P), np.float32)
    idx = np.arange(P)
    pswap[idx, idx ^ 1] = 1.0

    in_maps = []
    for c in range(8):
        b, rk = c // TP, c % TP
        in_maps.append({
            "xt": np.ascontiguousarray(x[b].T),
            "wq": np.ascontiguousarray(wq[:, rk * QH * HD:(rk + 1) * QH * HD]),
            "wk": np.ascontiguousarray(wk[:, rk * G * HD:(rk + 1) * G * HD]),
            "wv": np.ascontiguousarray(wv[:, rk * G * HD:(rk + 1) * G * HD]),
            "wo": np.ascontiguousarray(wo[rk * QH * HD:(rk + 1) * QH * HD, :]),
            "cost": cosT, "sint": sinT, "mbig": mbig,
            "onec": onec, "oner": oner, "pswap": pswap,
        })
    return in_maps


def kernel(x, wq, wk, wv, wo, cos, sin, mask=None, positions=None, **_):
    global LAST_EXEC_NS, LAST_TRACE_DIR
    x = np.asarray(x, np.float32)
    wq = np.asarray(wq, np.float32)
    wk = np.asarray(wk, np.float32)
    wv = np.asarray(wv, np.float32)
    wo = np.asarray(wo, np.float32)
    cos = np.asarray(cos, np.float32)
    sin = np.asarray(sin, np.float32)

    sys.path.insert(0, "/opt/trn_rl_repo")
    from concourse.bass_utils import run_bass_kernel_spmd

    nc = _build()
    in_maps = _prep_in_maps(x, wq, wk, wv, wo, cos, sin)
    trace = bool(int(os.environ.get("BASS_TRACE", "0") or "0"))
    res = run_bass_kernel_spmd(nc, in_maps, list(range(8)), trace=trace)
    LAST_EXEC_NS = res.exec_time_ns
    if LAST_EXEC_NS is None and os.environ.get("BASS_WALLTIME", "1") == "1":
        import time as _time
        t0 = _time.perf_counter()
        res = run_bass_kernel_spmd(nc, in_maps, list(range(8)), trace=False)
        LAST_EXEC_NS = int((_time.perf_counter() - t0) * 1e9)
    try:
        LAST_TRACE_DIR = getattr(res, "profile_json", None)
    except Exception:
        LAST_TRACE_DIR = None

    out = np.empty((B, S, D), np.float32)
    if os.environ.get("KERNEL_NO_CC", "0") == "1":
        for b in range(B):
            out[b] = sum(res.results[b * TP + rk]["out"] for rk in range(TP))
    else:
        for c in range(8):
            b, rk = c // TP, c % TP
            o = res.results[c]["out"]
            for t in range(NQT):
                out[b, t * QT + rk * P: t * QT + (rk + 1) * P, :] = \
                    o[t * P:(t + 1) * P, :]
    return out
